# revision 7
# baseline (speedup 1.0000x reference)
"""TRN2 Bass kernel for nn_Network_70918499991665 (implicit-surface MLP fwd + input-Jacobian).

Structure: 12-layer MLP over N=4096 points (B=1), each layer y = softplus_beta(W x + b)
with analytic input-gradient propagation g_out = sigmoid(beta*pre) * (W g_in).
Points are sharded across 8 NeuronCores (512 points/core); weights replicated.

Per core, the layer state Z_k is kept in SBUF as [C_k, 4*512] fp32(r): columns are
[x | g_x | g_y | g_z] chunks of 512 points. Each layer is one tiled matmul
pre = W_k @ Z_{k-1} (PE, f32r: 1 cyc/row) followed by the activation epilogue
built from the single ACT LUT set {relu, abs, exp, ln, identity}:
  x_out = relu(t) + 0.01*ln(1 + exp(-100*|t|)),  t = pre_x + b
  sg    = exp(100*(pre_x - x_out) + 100*b)          (== sigmoid(100 t), safe)
  g_out = sg * pre_g
Weight-norm (w = g*v/||v||) is host-side in float64; weight slabs are host-packed
into the SBUF layout [128, ci_tiles*Cout_pad] so each layer loads with one DMA.
"""

import numpy as np
from contextlib import ExitStack

BETA = 100.0
NCORES = 8
NPTS = 4096
NLOC = NPTS // NCORES  # 512 points per core
P = 128

# (param_prefix, Cin, Cout, n_branches) -- branches concatenated into one [Cout, Cin] W
_LAYERS = [
    ("c1", 259, 515, None),
    ("r1", 515, 512, (16, 32)),
    ("r1_1", 512, 512, (16, 32)),
    ("r2", 512, 576, (12, 48)),
    ("r2_2", 576, 576, (12, 48)),
    ("r3", 576, 768, (12, 64)),
    ("r3_3", 768, 768, (8, 96)),
    ("r4", 768, 768, (6, 128)),
    ("r4_4", 768, 960, (6, 160)),
    ("r5", 960, 960, (5, 192)),
    ("r5_5", 960, 896, (4, 224)),
    ("c10", 896, 1, None),
]


def _ceil_div(a, b):
    return (a + b - 1) // b


def _wn64(v, g):
    v = v.astype(np.float64)
    g = g.astype(np.float64)
    n = np.sqrt(np.sum(v * v, axis=-1, keepdims=True))
    return g[..., None] * v / n


def _host_weights(inputs):
    """Returns per-layer (slab[128, ci_t*Cout_pad] f32, bias[Cout] f64) plus w1 [515, 259]."""
    out = []
    w1 = None
    for name, cin, cout, br in _LAYERS:
        v = np.asarray(inputs[name + "_v"])
        g = np.asarray(inputs[name + "_g"])
        b = np.asarray(inputs[name + "_b"]).astype(np.float64).reshape(-1)
        w = _wn64(v, g)  # [L,c,cin] or [cout, cin]
        w = w.reshape(-1, cin)  # [Cout, Cin]
        if name == "c1":
            w1 = w
        ci_t = _ceil_div(cin, P)
        co_t = _ceil_div(cout, P)
        cop = co_t * P
        wt = np.zeros((ci_t * P, cop), np.float64)
        wt[:cin, :cout] = w.T
        slab = (
            wt.reshape(ci_t, P, cop).transpose(1, 0, 2).reshape(P, ci_t * cop)
        ).astype(np.float32)
        out.append((slab, b))
    return out, w1


def _build_program(mm_dtype_name):
    import concourse.bass as bass
    import concourse.tile as tile
    from concourse import mybir

    AF = mybir.ActivationFunctionType
    DT = getattr(mybir.dt, mm_dtype_name)
    F32 = mybir.dt.float32

    specs = []
    for name, cin, cout, _ in _LAYERS:
        specs.append((name, cin, cout, _ceil_div(cin, P), _ceil_div(cout, P)))

    # bias slab column layout: per main layer co_t cols of b, then co_t cols of 100b;
    # then 15 cols of w1col (5 co-tiles x 3 dims); then 1 col c10_b.
    bcol = {}
    col = 0
    for name, cin, cout, ci_t, co_t in specs[:-1]:
        bcol[name] = col
        col += co_t
        bcol[name + "_100"] = col
        col += co_t
    bcol["w1col"] = col
    col += specs[0][4] * 3
    bcol["c10b"] = col
    col += 1
    BCOLS = col

    nc = bass.Bass()
    x0_d = nc.declare_dram_parameter("x0", [P, 3 * NLOC], DT, isOutput=False)
    w_d = {}
    for name, cin, cout, ci_t, co_t in specs:
        w_d[name] = nc.declare_dram_parameter(
            "w_" + name, [P, ci_t * co_t * P], DT, isOutput=False
        )
    bias_d = nc.declare_dram_parameter("biases", [P, BCOLS], F32, isOutput=False)
    res_d = nc.declare_dram_parameter("res", [1, 4 * NLOC], F32, isOutput=True)

    with tile.TileContext(nc) as tc:
        with ExitStack() as ctx:
            zpool = ctx.enter_context(tc.tile_pool(name="z", bufs=14))
            wpool = ctx.enter_context(tc.tile_pool(name="w", bufs=1))
            bpool = ctx.enter_context(tc.tile_pool(name="b", bufs=1))
            rpool = ctx.enter_context(tc.tile_pool(name="tr", bufs=2))
            apool = ctx.enter_context(tc.tile_pool(name="ta", bufs=2))
            xpool = ctx.enter_context(tc.tile_pool(name="tx", bufs=2))
            dpool = ctx.enter_context(tc.tile_pool(name="td", bufs=2))
            spool = ctx.enter_context(tc.tile_pool(name="ts", bufs=2))
            pspool = ctx.enter_context(tc.tile_pool(name="ps", bufs=2, space="PSUM"))

            bias_t = bpool.tile([P, BCOLS], F32)
            nc.sync.dma_start(bias_t[:], bias_d[:])

            # x0: 3 ci tiles of [128, 512]
            z_in = []
            for ci in range(3):
                zt = zpool.tile([P, NLOC], DT, tag="z")
                nc.sync.dma_start(zt[:], x0_d[:, ci * NLOC : (ci + 1) * NLOC])
                z_in.append(zt)

            for li, (name, cin, cout, ci_t, co_t) in enumerate(specs):
                cop = co_t * P
                first = li == 0
                last = li == len(specs) - 1
                wt = wpool.tile([P, ci_t * cop], DT, tag=f"w{li % 2}")
                nc.sync.dma_start(wt[:], w_d[name][:])

                if last:
                    # c10: Cout=1, no activation; emit 4 chunks into one psum tile
                    ps = pspool.tile([P, 4 * NLOC], F32)
                    for f in range(4):
                        for ci in range(ci_t):
                            nc.tensor.matmul(
                                ps[0:1, f * NLOC : (f + 1) * NLOC],
                                wt[:, ci * cop : ci * cop + 1],
                                z_in[ci][:, f * NLOC : (f + 1) * NLOC],
                                start=(ci == 0),
                                stop=(ci == ci_t - 1),
                            )
                    out_sb = zpool.tile([1, 4 * NLOC], F32, tag="z")
                    c10b_ap = bias_t[0:1, bcol["c10b"] : bcol["c10b"] + 1]
                    nc.scalar.activation(
                        out_sb[0:1, 0:NLOC], ps[0:1, 0:NLOC], AF.Identity, bias=c10b_ap
                    )
                    for f in range(1, 4):
                        nc.scalar.activation(
                            out_sb[0:1, f * NLOC : (f + 1) * NLOC],
                            ps[0:1, f * NLOC : (f + 1) * NLOC],
                            AF.Identity,
                        )
                    nc.sync.dma_start(res_d[:], out_sb[:])
                    break

                nchunk = 1 if first else 4
                z_out = []
                for co in range(co_t):
                    ps = pspool.tile([P, 4 * NLOC], F32)
                    for f in range(nchunk):
                        for ci in range(ci_t):
                            nc.tensor.matmul(
                                ps[:, f * NLOC : (f + 1) * NLOC],
                                wt[:, ci * cop + co * P : ci * cop + (co + 1) * P],
                                z_in[ci][:, f * NLOC : (f + 1) * NLOC],
                                start=(ci == 0),
                                stop=(ci == ci_t - 1),
                            )
                    zo = zpool.tile([P, 4 * NLOC], DT, tag="z")
                    b_ap = bias_t[:, bcol[name] + co : bcol[name] + co + 1]
                    b100_ap = bias_t[
                        :, bcol[name + "_100"] + co : bcol[name + "_100"] + co + 1
                    ]
                    P0 = ps[:, 0:NLOC]
                    r = rpool.tile([P, NLOC], F32)
                    nc.scalar.activation(r[:], P0, AF.Relu, bias=b_ap)
                    a = apool.tile([P, NLOC], F32)
                    nc.scalar.activation(a[:], P0, AF.Abs, bias=b_ap)
                    nc.scalar.activation(a[:], a[:], AF.Exp, scale=-BETA)
                    nc.scalar.activation(a[:], a[:], AF.Ln, bias=1.0)
                    nc.vector.tensor_scalar_mul(a[:], a[:], 1.0 / BETA)
                    xo = xpool.tile([P, NLOC], F32)
                    nc.vector.tensor_add(xo[:], r[:], a[:])
                    nc.vector.tensor_copy(zo[:, 0:NLOC], xo[:])
                    dd = dpool.tile([P, NLOC], F32)
                    nc.vector.tensor_sub(dd[:], P0, xo[:])
                    sg = spool.tile([P, NLOC], F32)
                    nc.scalar.activation(sg[:], dd[:], AF.Exp, scale=BETA, bias=b100_ap)
                    if first:
                        for d in range(3):
                            wc = bcol["w1col"] + co * 3 + d
                            nc.vector.tensor_scalar_mul(
                                zo[:, (d + 1) * NLOC : (d + 2) * NLOC],
                                sg[:],
                                bias_t[:, wc : wc + 1],
                            )
                    else:
                        for d in range(3):
                            nc.vector.tensor_mul(
                                zo[:, (d + 1) * NLOC : (d + 2) * NLOC],
                                sg[:],
                                ps[:, (d + 1) * NLOC : (d + 2) * NLOC],
                            )
                    z_out.append(zo)
                z_in = z_out

    _split_embedded_waits(nc, mybir)
    return nc, bcol, BCOLS


def _split_embedded_waits(nc, mybir, limit=1):
    """Walrus codegen allows very few embedded sem-waits per instruction on cayman
    (f32r matmuls and drains overflow). Hoist excess waits onto same-engine NoOps."""
    ctr = 0
    for fn in nc.m.functions:
        for bb in fn.blocks:
            out = []
            for inst in bb.instructions:
                si = inst.sync_info
                waits = list(si.on_wait) if si is not None and si.on_wait else []
                if len(waits) > limit:
                    keep = waits[-limit:]
                    excess = waits[:-limit]
                    for i in range(0, len(excess), limit):
                        chunk = excess[i : i + limit]
                        ctr += 1
                        out.append(
                            mybir.InstNoOp(
                                name=f"I-wsplit-{ctr}",
                                ins=[],
                                outs=[],
                                engine=inst.engine,
                                sync_info=mybir.SyncInfo(
                                    on_wait=list(chunk), on_update=[]
                                ),
                                bass_nofuse=True,
                            )
                        )
                    inst.sync_info = mybir.SyncInfo(
                        on_wait=keep, on_update=list(si.on_update)
                    )
                out.append(inst)
            bb.instructions = out


_CACHE = {}

MM_DTYPE = "float32r"  # "float32r" (TF32-class, 4x faster) or "float32" (exact)


def _get_program():
    key = MM_DTYPE
    if key not in _CACHE:
        _CACHE[key] = _build_program(key)
    return _CACHE[key]


def _host_inputs(inputs, bcol, BCOLS):
    wb, w1 = _host_weights(inputs)
    base = {}
    for (name, cin, cout, _), (slab, b) in zip(_LAYERS, wb):
        base["w_" + name] = slab
    # bias slab
    bias = np.zeros((P, BCOLS), np.float64)
    for (name, cin, cout, _), (slab, b) in zip(_LAYERS[:-1], wb[:-1]):
        co_t = _ceil_div(cout, P)
        bp = np.zeros(co_t * P)
        bp[:cout] = b
        cols = bp.reshape(co_t, P).T  # [128, co_t]
        bias[:, bcol[name] : bcol[name] + co_t] = cols
        bias[:, bcol[name + "_100"] : bcol[name + "_100"] + co_t] = BETA * cols
    # w1col: first 3 columns of normalized c1 weight, [515]->pad 640, per co-tile
    co1 = _ceil_div(_LAYERS[0][2], P)
    w1p = np.zeros((co1 * P, 3))
    w1p[: w1.shape[0], :] = w1[:, :3]
    bias[:, bcol["w1col"] : bcol["w1col"] + co1 * 3] = (
        w1p.reshape(co1, P, 3).transpose(1, 0, 2).reshape(P, co1 * 3)
    )
    bias[0, bcol["c10b"]] = float(np.asarray(inputs["c10_b"]).reshape(-1)[0])
    base["biases"] = bias.astype(np.float32)

    # per-core x0: [128, 3*512]: channels (3 xyz + 256 latent, pad to 384) x points
    inp = np.asarray(inputs["input"]).astype(np.float32)  # [1, 4096, 3]
    lat = np.asarray(inputs["latent"]).astype(np.float32)  # [1, 256]
    in_maps = []
    for c in range(NCORES):
        sl = slice(c * NLOC, (c + 1) * NLOC)
        x0 = np.zeros((3 * P, NLOC), np.float32)
        x0[0:3, :] = inp[0, sl, :].T
        x0[3:259, :] = lat[0][:, None]
        x0 = x0.reshape(3, P, NLOC).transpose(1, 0, 2).reshape(P, 3 * NLOC)
        m = dict(base)
        m["x0"] = np.ascontiguousarray(x0)
        in_maps.append(m)
    return in_maps


def kernel(**inputs):
    from concourse.bass_utils import run_bass_kernel_spmd

    nc, bcol, BCOLS = _get_program()
    in_maps = _host_inputs(inputs, bcol, BCOLS)
    res = run_bass_kernel_spmd(nc, in_maps, list(range(NCORES)))

    out = np.zeros((1, 1, NPTS), np.float32)
    out_grad = np.zeros((1, NPTS, 1, 3), np.float32)
    for c in range(NCORES):
        r = res.results[c]["res"][0]
        sl = slice(c * NLOC, (c + 1) * NLOC)
        out[0, 0, sl] = r[0:NLOC]
        for d in range(3):
            out_grad[0, sl, 0, d] = r[(d + 1) * NLOC : (d + 2) * NLOC]

    inp = np.asarray(inputs["input"]).astype(np.float32)
    lat = np.asarray(inputs["latent"]).astype(np.float32)
    input_con = np.concatenate(
        [inp, np.broadcast_to(lat[:, None, :], (1, NPTS, lat.shape[-1]))], axis=-1
    ).astype(np.float32)
    return out, out_grad, input_con


# revision 12
# speedup vs baseline: 1.2404x; 1.2404x over previous
"""TRN2 Bass kernel for nn_Network_70918499991665 (implicit-surface MLP fwd + input-Jacobian).

Structure: 12-layer MLP over N=4096 points (B=1), each layer y = softplus_beta(W x + b)
with analytic input-gradient propagation g_out = sigmoid(beta*pre) * (W g_in).
Points are sharded across 8 NeuronCores (512 points/core); weights replicated.

Per core, the layer state Z_k is kept in SBUF as [C_k, 4*512] fp32(r): columns are
[x | g_x | g_y | g_z] chunks of 512 points. Each layer is one tiled matmul
pre = W_k @ Z_{k-1} (PE, f32r: 1 cyc/row) followed by the activation epilogue
built from the single ACT LUT set {relu, abs, exp, ln, identity}:
  x_out = relu(t) + 0.01*ln(1 + exp(-100*|t|)),  t = pre_x + b
  sg    = exp(100*(pre_x - x_out) + 100*b)          (== sigmoid(100 t), safe)
  g_out = sg * pre_g
Weight-norm (w = g*v/||v||) is host-side in float64; weight slabs are host-packed
into the SBUF layout [128, ci_tiles*Cout_pad] so each layer loads with one DMA.
"""

import numpy as np
from contextlib import ExitStack

BETA = 100.0
NCORES = 8
NPTS = 4096
NLOC = NPTS // NCORES  # 512 points per core
P = 128

# (param_prefix, Cin, Cout, n_branches) -- branches concatenated into one [Cout, Cin] W
_LAYERS = [
    ("c1", 259, 515, None),
    ("r1", 515, 512, (16, 32)),
    ("r1_1", 512, 512, (16, 32)),
    ("r2", 512, 576, (12, 48)),
    ("r2_2", 576, 576, (12, 48)),
    ("r3", 576, 768, (12, 64)),
    ("r3_3", 768, 768, (8, 96)),
    ("r4", 768, 768, (6, 128)),
    ("r4_4", 768, 960, (6, 160)),
    ("r5", 960, 960, (5, 192)),
    ("r5_5", 960, 896, (4, 224)),
    ("c10", 896, 1, None),
]


def _ceil_div(a, b):
    return (a + b - 1) // b


def _wn64(v, g):
    v = v.astype(np.float64)
    g = g.astype(np.float64)
    n = np.sqrt(np.sum(v * v, axis=-1, keepdims=True))
    return g[..., None] * v / n


def _host_weights(inputs):
    """Returns per-layer (slab[128, ci_t*Cout_pad] f32, bias[Cout] f64) plus w1 [515, 259]."""
    out = []
    w1 = None
    for name, cin, cout, br in _LAYERS:
        v = np.asarray(inputs[name + "_v"])
        g = np.asarray(inputs[name + "_g"])
        b = np.asarray(inputs[name + "_b"]).astype(np.float64).reshape(-1)
        w = _wn64(v, g)  # [L,c,cin] or [cout, cin]
        w = w.reshape(-1, cin)  # [Cout, Cin]
        if name == "c1":
            w1 = w
        ci_t = _ceil_div(cin, P)
        co_t = _ceil_div(cout, P)
        cop = co_t * P
        wt = np.zeros((ci_t * P, cop), np.float64)
        wt[:cin, :cout] = w.T
        slab = (
            wt.reshape(ci_t, P, cop).transpose(1, 0, 2).reshape(P, ci_t * cop)
        ).astype(np.float32)
        out.append((slab, b))
    return out, w1


def _build_program(mm_dtype_name):
    import concourse.bass as bass
    import concourse.tile as tile
    from concourse import mybir

    AF = mybir.ActivationFunctionType
    DT = getattr(mybir.dt, mm_dtype_name)
    F32 = mybir.dt.float32

    specs = []
    for name, cin, cout, _ in _LAYERS:
        specs.append((name, cin, cout, _ceil_div(cin, P), _ceil_div(cout, P)))

    # bias slab column layout: per main layer co_t cols of b, then co_t cols of 100b;
    # then 15 cols of w1col (5 co-tiles x 3 dims); then 1 col c10_b.
    bcol = {}
    col = 0
    for name, cin, cout, ci_t, co_t in specs[:-1]:
        bcol[name] = col
        col += co_t
        bcol[name + "_100"] = col
        col += co_t
    bcol["w1col"] = col
    col += specs[0][4] * 3
    bcol["c10b"] = col
    col += 1
    BCOLS = col

    nc = bass.Bass()
    x0_d = nc.declare_dram_parameter("x0", [P, 3 * NLOC], DT, isOutput=False)
    w_d = {}
    for name, cin, cout, ci_t, co_t in specs:
        w_d[name] = nc.declare_dram_parameter(
            "w_" + name, [P, ci_t * co_t * P], DT, isOutput=False
        )
    bias_d = nc.declare_dram_parameter("biases", [P, BCOLS], F32, isOutput=False)
    res_d = nc.declare_dram_parameter("res", [1, 4 * NLOC], F32, isOutput=True)

    with tile.TileContext(nc) as tc:
        with ExitStack() as ctx:
            zpool = ctx.enter_context(tc.tile_pool(name="z", bufs=14))
            wpool = ctx.enter_context(tc.tile_pool(name="w", bufs=1))
            bpool = ctx.enter_context(tc.tile_pool(name="b", bufs=1))
            rpool = ctx.enter_context(tc.tile_pool(name="tr", bufs=3))
            apool = ctx.enter_context(tc.tile_pool(name="ta", bufs=3))
            xpool = ctx.enter_context(tc.tile_pool(name="tx", bufs=3))
            spool = ctx.enter_context(tc.tile_pool(name="ts", bufs=3))
            pspool = ctx.enter_context(tc.tile_pool(name="ps", bufs=2, space="PSUM"))

            bias_t = bpool.tile([P, BCOLS], F32)
            nc.sync.dma_start(bias_t[:], bias_d[:])

            # x0: 3 ci tiles of [128, 512]
            z_in = []
            for ci in range(3):
                zt = zpool.tile([P, NLOC], DT, tag="z")
                nc.sync.dma_start(zt[:], x0_d[:, ci * NLOC : (ci + 1) * NLOC])
                z_in.append(zt)

            for li, (name, cin, cout, ci_t, co_t) in enumerate(specs):
                cop = co_t * P
                first = li == 0
                last = li == len(specs) - 1
                wt = wpool.tile([P, ci_t * cop], DT, tag=f"w{li % 2}")
                nc.sync.dma_start(wt[:], w_d[name][:])

                if last:
                    # c10: Cout=1, no activation; emit 4 chunks into one psum tile
                    ps = pspool.tile([P, 4 * NLOC], F32)
                    for f in range(4):
                        for ci in range(ci_t):
                            nc.tensor.matmul(
                                ps[0:1, f * NLOC : (f + 1) * NLOC],
                                wt[:, ci * cop : ci * cop + 1],
                                z_in[ci][:, f * NLOC : (f + 1) * NLOC],
                                start=(ci == 0),
                                stop=(ci == ci_t - 1),
                            )
                    out_sb = zpool.tile([1, 4 * NLOC], F32, tag="z")
                    c10b_ap = bias_t[0:1, bcol["c10b"] : bcol["c10b"] + 1]
                    nc.scalar.activation(
                        out_sb[0:1, 0:NLOC], ps[0:1, 0:NLOC], AF.Identity, bias=c10b_ap
                    )
                    for f in range(1, 4):
                        nc.scalar.activation(
                            out_sb[0:1, f * NLOC : (f + 1) * NLOC],
                            ps[0:1, f * NLOC : (f + 1) * NLOC],
                            AF.Identity,
                        )
                    nc.sync.dma_start(res_d[:], out_sb[:])
                    break

                nchunk = 1 if first else 4
                z_out = []
                for co in range(co_t):
                    ps = pspool.tile([P, 4 * NLOC], F32)
                    for f in range(nchunk):
                        for ci in range(ci_t):
                            nc.tensor.matmul(
                                ps[:, f * NLOC : (f + 1) * NLOC],
                                wt[:, ci * cop + co * P : ci * cop + (co + 1) * P],
                                z_in[ci][:, f * NLOC : (f + 1) * NLOC],
                                start=(ci == 0),
                                stop=(ci == ci_t - 1),
                            )
                    zo = zpool.tile([P, 4 * NLOC], DT, tag="z")
                    b_ap = bias_t[:, bcol[name] + co : bcol[name] + co + 1]
                    P0 = ps[:, 0:NLOC]
                    # th = max(pre + b, -0.85); below -0.85 both softplus_b and
                    # sigmoid(100t) underflow to 0, so the clamp is exact in fp32.
                    th = rpool.tile([P, NLOC], F32)
                    nc.vector.tensor_scalar(
                        th[:], P0, b_ap, -0.85, mybir.AluOpType.add, mybir.AluOpType.max
                    )
                    # u' = exp(-100 th); l = ln(1+u') = 100*softplus_b - 100*relu;
                    # sg = exp(-l) = 1/(1+u') = sigmoid(100 th) to fp32 accuracy.
                    u = apool.tile([P, NLOC], F32)
                    nc.scalar.activation(u[:], th[:], AF.Exp, scale=-BETA)
                    l = xpool.tile([P, NLOC], F32)
                    nc.scalar.activation(l[:], u[:], AF.Ln, bias=1.0)
                    sg = spool.tile([P, NLOC], F32)
                    nc.scalar.activation(sg[:], l[:], AF.Exp, scale=-1.0)
                    # x_out = th + 0.01*l  (fused on GpSimd, written as fp16/f32r)
                    nc.gpsimd.scalar_tensor_tensor(
                        zo[:, 0:NLOC],
                        l[:],
                        1.0 / BETA,
                        th[:],
                        mybir.AluOpType.mult,
                        mybir.AluOpType.add,
                    )
                    if first:
                        for d in range(3):
                            wc = bcol["w1col"] + co * 3 + d
                            nc.vector.tensor_scalar_mul(
                                zo[:, (d + 1) * NLOC : (d + 2) * NLOC],
                                sg[:],
                                bias_t[:, wc : wc + 1],
                            )
                    else:
                        for d in range(3):
                            nc.vector.tensor_mul(
                                zo[:, (d + 1) * NLOC : (d + 2) * NLOC],
                                sg[:],
                                ps[:, (d + 1) * NLOC : (d + 2) * NLOC],
                            )
                    z_out.append(zo)
                z_in = z_out

    _split_embedded_waits(nc, mybir)
    return nc, bcol, BCOLS


def _split_embedded_waits(nc, mybir, limit=1):
    """Walrus codegen allows very few embedded sem-waits per instruction on cayman
    (f32r matmuls and drains overflow). Hoist excess waits onto same-engine NoOps."""
    ctr = 0
    for fn in nc.m.functions:
        for bb in fn.blocks:
            out = []
            for inst in bb.instructions:
                si = inst.sync_info
                waits = list(si.on_wait) if si is not None and si.on_wait else []
                if len(waits) > limit:
                    keep = waits[-limit:]
                    excess = waits[:-limit]
                    for i in range(0, len(excess), limit):
                        chunk = excess[i : i + limit]
                        ctr += 1
                        out.append(
                            mybir.InstNoOp(
                                name=f"I-wsplit-{ctr}",
                                ins=[],
                                outs=[],
                                engine=inst.engine,
                                sync_info=mybir.SyncInfo(
                                    on_wait=list(chunk), on_update=[]
                                ),
                                bass_nofuse=True,
                            )
                        )
                    inst.sync_info = mybir.SyncInfo(
                        on_wait=keep, on_update=list(si.on_update)
                    )
                out.append(inst)
            bb.instructions = out


_CACHE = {}

import os as _os

# "float16": ~10-bit-mantissa matmuls, weight loads overlap (fastest)
# "float32r": ~11-bit-mantissa, weight load serializes with matmul (~2x slower)
# "float32": exact, 4 cyc/row (~4x slower)
MM_DTYPE = _os.environ.get("MM_DTYPE", "float16")


def _get_program():
    key = MM_DTYPE
    if key not in _CACHE:
        _CACHE[key] = _build_program(key)
    return _CACHE[key]


def _host_inputs(inputs, bcol, BCOLS):
    np_dt = np.float16 if MM_DTYPE == "float16" else np.float32
    wb, w1 = _host_weights(inputs)
    base = {}
    for (name, cin, cout, _), (slab, b) in zip(_LAYERS, wb):
        base["w_" + name] = slab.astype(np_dt)
    # bias slab
    bias = np.zeros((P, BCOLS), np.float64)
    for (name, cin, cout, _), (slab, b) in zip(_LAYERS[:-1], wb[:-1]):
        co_t = _ceil_div(cout, P)
        bp = np.zeros(co_t * P)
        bp[:cout] = b
        cols = bp.reshape(co_t, P).T  # [128, co_t]
        bias[:, bcol[name] : bcol[name] + co_t] = cols
        bias[:, bcol[name + "_100"] : bcol[name + "_100"] + co_t] = BETA * cols
    # w1col: first 3 columns of normalized c1 weight, [515]->pad 640, per co-tile
    co1 = _ceil_div(_LAYERS[0][2], P)
    w1p = np.zeros((co1 * P, 3))
    w1p[: w1.shape[0], :] = w1[:, :3]
    bias[:, bcol["w1col"] : bcol["w1col"] + co1 * 3] = (
        w1p.reshape(co1, P, 3).transpose(1, 0, 2).reshape(P, co1 * 3)
    )
    bias[0, bcol["c10b"]] = float(np.asarray(inputs["c10_b"]).reshape(-1)[0])
    base["biases"] = bias.astype(np.float32)

    # per-core x0: [128, 3*512]: channels (3 xyz + 256 latent, pad to 384) x points
    inp = np.asarray(inputs["input"]).astype(np.float32)  # [1, 4096, 3]
    lat = np.asarray(inputs["latent"]).astype(np.float32)  # [1, 256]
    in_maps = []
    for c in range(NCORES):
        sl = slice(c * NLOC, (c + 1) * NLOC)
        x0 = np.zeros((3 * P, NLOC), np.float32)
        x0[0:3, :] = inp[0, sl, :].T
        x0[3:259, :] = lat[0][:, None]
        x0 = x0.reshape(3, P, NLOC).transpose(1, 0, 2).reshape(P, 3 * NLOC)
        m = dict(base)
        m["x0"] = np.ascontiguousarray(x0).astype(np_dt)
        in_maps.append(m)
    return in_maps


def kernel(**inputs):
    from concourse.bass_utils import run_bass_kernel_spmd

    nc, bcol, BCOLS = _get_program()
    in_maps = _host_inputs(inputs, bcol, BCOLS)
    res = run_bass_kernel_spmd(nc, in_maps, list(range(NCORES)))

    out = np.zeros((1, 1, NPTS), np.float32)
    out_grad = np.zeros((1, NPTS, 1, 3), np.float32)
    for c in range(NCORES):
        r = res.results[c]["res"][0]
        sl = slice(c * NLOC, (c + 1) * NLOC)
        out[0, 0, sl] = r[0:NLOC]
        for d in range(3):
            out_grad[0, sl, 0, d] = r[(d + 1) * NLOC : (d + 2) * NLOC]

    inp = np.asarray(inputs["input"]).astype(np.float32)
    lat = np.asarray(inputs["latent"]).astype(np.float32)
    input_con = np.concatenate(
        [inp, np.broadcast_to(lat[:, None, :], (1, NPTS, lat.shape[-1]))], axis=-1
    ).astype(np.float32)
    return out, out_grad, input_con
